# revision 1
# baseline (speedup 1.0000x reference)
"""LSTM kernel for Trainium2 (Bass/Tile), SPMD over 8 NeuronCores.

Problem: B=128, S=1024, D=256, H=512, C=10 LSTM; output = final hidden state
projected to C classes -> [B, C].

Sharding: data-parallel over batch (16 per core); weights replicated;
recurrence local per shard (no collectives).

Per-core program (two phases, one kernel launch):
  Phase 1: proj[t] = x_t @ [Wgx|Wix|Wfx|Wox ; b] for all t via full-PE GEMM
           (x stationary [128,128] tiles, W moving fp32r at 1 cyc/row),
           written to a DRAM scratch tensor.
  Phase 2: sequential recurrence. Per step: 16 matmuls (hT stationary
           [128,16], Wh moving N=512 fp32r) accumulate h@Wh into PSUM;
           DVE adds proj; ACT applies tanh/sigmoid per gate-pure 512-chunk;
           DVE cell update; PE-transposes h back into stationary hT form.
"""

import numpy as np

S, B, D, H, C = 1024, 128, 256, 512, 10
NCORES = 8
BC = B // NCORES          # batch per core
G4 = 4 * H                # fused gate width
NK_H = H // 128           # 4 K-tiles for h
NK_X = D // 128 + 1       # 2 K-tiles for x + 1 bias tile
CHUNK = 512               # PSUM-bank-sized gate chunk (one gate each: G,I,F,O)
NJ = G4 // CHUNK          # 4 chunks


def _build_nc(s_total: int):
    import concourse.bass as bass
    import concourse.mybir as mybir
    import concourse.tile as tile
    from concourse import bacc
    from concourse.masks import make_identity

    f32 = mybir.dt.float32
    f32r = mybir.dt.float32r
    AF = mybir.ActivationFunctionType
    OP = mybir.AluOpType

    m_tiles = s_total * BC // 128     # phase-1 M tiles (8 steps each)
    r_blocks = s_total // 4           # phase-2 proj DMA blocks

    nc = bacc.Bacc(
        "TRN2",
        target_bir_lowering=False,
        debug=False,
        enable_asserts=False,
        num_devices=NCORES,
    )

    xT_d = nc.dram_tensor("xT", [m_tiles, 128, NK_X, 128], f32r, kind="ExternalInput").ap()
    Wx_d = nc.dram_tensor("Wx", [NK_X, 128, G4], f32r, kind="ExternalInput").ap()
    Wh_d = nc.dram_tensor("Wh", [NK_H, 128, G4], f32r, kind="ExternalInput").ap()
    Wp_d = nc.dram_tensor("Wp", [NK_H, 128, C], f32r, kind="ExternalInput").ap()
    bp_d = nc.dram_tensor("bp", [BC, C], f32, kind="ExternalInput").ap()
    h0_d = nc.dram_tensor("h0", [128, NK_H * BC], f32r, kind="ExternalInput").ap()
    out_d = nc.dram_tensor("out", [BC, C], f32, kind="ExternalOutput").ap()

    with tile.TileContext(nc) as tc:
        with (
            tc.tile_pool(name="const", bufs=1) as const,
            tc.tile_pool(name="state", bufs=1) as state,
            tc.tile_pool(name="dram", bufs=1, space="DRAM") as dram,
        ):
            Wx_sb = const.tile([128, NK_X * G4], f32r)
            nc.sync.dma_start(
                Wx_sb[:].rearrange("p (k g) -> p k g", k=NK_X),
                Wx_d.rearrange("k p g -> p k g"),
            )
            Wh_sb = const.tile([128, NK_H * G4], f32r)
            nc.sync.dma_start(
                Wh_sb[:].rearrange("p (k g) -> p k g", k=NK_H),
                Wh_d.rearrange("k p g -> p k g"),
            )
            Wp_sb = const.tile([128, NK_H * C], f32r)
            nc.sync.dma_start(
                Wp_sb[:].rearrange("p (k c) -> p k c", k=NK_H),
                Wp_d.rearrange("k p c -> p k c"),
            )
            bp_sb = const.tile([BC, C], f32)
            nc.sync.dma_start(bp_sb[:], bp_d[:])
            ident = const.tile([BC, BC], f32)
            make_identity(nc, ident[:])

            # Recurrent state, ping-pong. hT is the transposed hidden state
            # [H-row, batch] packed as 4 K-tiles side by side: hT[:, 16k:16k+16].
            hT = [state.tile([128, NK_H * BC], f32r, tag=f"hT{i}", name=f"hT{i}") for i in range(2)]
            cs = [state.tile([BC, H], f32, tag=f"c{i}", name=f"c{i}") for i in range(2)]
            nc.sync.dma_start(hT[0][:], h0_d[:])
            nc.gpsimd.memset(cs[0][:], 0.0)

            # proj row index = 16*t + b (t = timestep, b = local batch)
            proj = dram.tile([s_total * BC, G4], f32)

            # ---------------- Phase 1: input projections ----------------
            with (
                tc.tile_pool(name="p1x", bufs=3) as p1x,
                tc.tile_pool(name="p1ps", bufs=2, space="PSUM") as p1ps,
                tc.tile_pool(name="p1st", bufs=3) as p1st,
            ):
                for m in range(m_tiles):
                    xt = p1x.tile([128, NK_X * 128], f32r)
                    nc.sync.dma_start(xt[:], xT_d[m].rearrange("p k c -> p (k c)"))
                    ps = p1ps.tile([128, G4], f32)
                    for j in range(NJ):
                        for k in range(NK_X):
                            nc.tensor.matmul(
                                ps[:, j * CHUNK:(j + 1) * CHUNK],
                                lhsT=xt[:, k * 128:(k + 1) * 128],
                                rhs=Wx_sb[:, k * G4 + j * CHUNK: k * G4 + (j + 1) * CHUNK],
                                start=(k == 0),
                                stop=(k == NK_X - 1),
                            )
                    st = p1st.tile([128, G4], f32)
                    for j in range(NJ):
                        src = ps[:, j * CHUNK:(j + 1) * CHUNK]
                        dst = st[:, j * CHUNK:(j + 1) * CHUNK]
                        if j % 2 == 0:
                            nc.vector.tensor_copy(dst, src)
                        else:
                            nc.scalar.copy(dst, src)
                    # m-tile covers steps 8m..8m+7 = proj blocks 2m, 2m+1;
                    # sbuf partition p = (t-8m)*16 + b matches (Blk s b) order.
                    nc.sync.dma_start(proj[128 * m:128 * (m + 1), :], st[:])

            # ---------------- Phase 2: recurrence ----------------
            with (
                tc.tile_pool(name="p2pj", bufs=2) as p2pj,
                tc.tile_pool(name="p2ps", bufs=1, space="PSUM") as p2ps,
                tc.tile_pool(name="p2tr", bufs=2, space="PSUM") as p2tr,
                tc.tile_pool(name="p2g", bufs=2) as p2g,
                tc.tile_pool(name="p2t", bufs=2) as p2t,
                tc.tile_pool(name="p2o", bufs=1, space="PSUM") as p2o,
            ):
                for r in range(r_blocks):
                    pj = p2pj.tile([BC, 4 * G4], f32)
                    nc.sync.dma_start(
                        pj[:].rearrange("b (s f) -> b s f", s=4),
                        proj[64 * r:64 * (r + 1), :].rearrange("(s b) f -> b s f", s=4),
                    )
                    for sidx in range(4):
                        t = 4 * r + sidx
                        cur, nxt = t % 2, (t + 1) % 2
                        ps = p2ps.tile([BC, G4], f32)
                        for j in range(NJ):
                            for k in range(NK_H):
                                nc.tensor.matmul(
                                    ps[:, j * CHUNK:(j + 1) * CHUNK],
                                    lhsT=hT[cur][:, k * BC:(k + 1) * BC],
                                    rhs=Wh_sb[:, k * G4 + j * CHUNK: k * G4 + (j + 1) * CHUNK],
                                    start=(k == 0),
                                    stop=(k == NK_H - 1),
                                )
                        gates = []
                        for j in range(NJ):
                            pre = p2t.tile([BC, CHUNK], f32, tag="pre", name="pre")
                            nc.vector.scalar_tensor_tensor(
                                pre[:],
                                ps[:, j * CHUNK:(j + 1) * CHUNK],
                                1.0,
                                pj[:, sidx * G4 + j * CHUNK: sidx * G4 + (j + 1) * CHUNK],
                                op0=OP.mult,
                                op1=OP.add,
                            )
                            gate = p2g.tile([BC, CHUNK], f32, tag=f"gate{j}", name=f"gate{j}")
                            nc.scalar.activation(
                                gate[:], pre[:],
                                AF.Tanh if j == 0 else AF.Sigmoid,
                            )
                            gates.append(gate)
                        g_, i_, f_, o_ = gates
                        gi = p2t.tile([BC, H], f32, tag="gi", name="gi")
                        nc.vector.tensor_mul(gi[:], g_[:], i_[:])
                        cn = cs[nxt]
                        nc.vector.tensor_mul(cn[:], cs[cur][:], f_[:])
                        nc.vector.tensor_add(cn[:], cn[:], gi[:])
                        th = p2t.tile([BC, H], f32, tag="th", name="th")
                        nc.scalar.activation(th[:], cn[:], AF.Tanh)
                        hn = p2t.tile([BC, H], f32, tag="hn", name="hn")
                        nc.vector.tensor_mul(hn[:], th[:], o_[:])
                        tr = p2tr.tile([128, NK_H * BC], f32)
                        for k in range(NK_H):
                            nc.tensor.transpose(
                                tr[:, k * BC:(k + 1) * BC],
                                hn[:, k * 128:(k + 1) * 128],
                                ident[:],
                            )
                        nc.vector.tensor_copy(hT[nxt][:], tr[:])

                # Final projection: out = h_S @ Wp + bp
                fin = s_total % 2
                pso = p2o.tile([BC, C], f32)
                for k in range(NK_H):
                    nc.tensor.matmul(
                        pso[:],
                        lhsT=hT[fin][:, k * BC:(k + 1) * BC],
                        rhs=Wp_sb[:, k * C:(k + 1) * C],
                        start=(k == 0),
                        stop=(k == NK_H - 1),
                    )
                res = p2g.tile([BC, C], f32, tag="res", name="res")
                nc.vector.tensor_add(res[:], pso[:], bp_sb[:])
                nc.sync.dma_start(out_d[:], res[:])

    nc.compile()
    return nc


def _prep_core_inputs(x, Wx_all, b_all, Wh_all, Wp, bp, core, s_total):
    """Build per-core numpy input map. x: [B, S, D] full batch."""
    m_tiles = s_total * BC // 128
    b0 = core * BC
    xc = x[b0:b0 + BC, :s_total, :]                     # [BC, s, D]
    # xT_host[m, p, kx, c]: stationary tiles; col c = (t - 8m)*16 + b
    a = np.ascontiguousarray(xc.transpose(2, 1, 0))     # [D, s, BC]
    a = a.reshape(D // 128, 128, m_tiles, 8, BC)        # [kx, p, m, t8, b]
    a = a.transpose(2, 1, 0, 3, 4).reshape(m_tiles, 128, D // 128, 128)
    xT = np.zeros((m_tiles, 128, NK_X, 128), dtype=np.float32)
    xT[:, :, :D // 128, :] = a
    xT[:, 0, NK_X - 1, :] = 1.0                          # bias ones-row
    return {"xT": np.ascontiguousarray(xT)}


def _prep_shared_inputs(Wgx, Wix, Wfx, Wox, Wgh, Wih, Wfh, Woh, bg, bi, bf, bo, Wph, bp):
    Wx_all = np.concatenate([Wgx, Wix, Wfx, Wox], axis=1).astype(np.float32)  # [D, G4]
    b_all = np.concatenate([bg, bi, bf, bo]).astype(np.float32)               # [G4]
    Wh_all = np.concatenate([Wgh, Wih, Wfh, Woh], axis=1).astype(np.float32)  # [H, G4]

    Wx = np.zeros((NK_X, 128, G4), dtype=np.float32)
    Wx[:D // 128] = Wx_all.reshape(D // 128, 128, G4)
    Wx[NK_X - 1, 0, :] = b_all                           # bias row (pairs with ones-row)
    Wh = np.ascontiguousarray(Wh_all.reshape(NK_H, 128, G4))
    Wp = np.ascontiguousarray(Wph.reshape(NK_H, 128, C).astype(np.float32))
    bpr = np.broadcast_to(bp.astype(np.float32), (BC, C)).copy()
    return Wx, Wh, Wp, bpr, Wx_all, b_all, Wh_all


_NC_CACHE = {}


def _get_nc(s_total):
    if s_total not in _NC_CACHE:
        _NC_CACHE[s_total] = _build_nc(s_total)
    return _NC_CACHE[s_total]


def kernel(x, Wgx, Wix, Wfx, Wox, Wgh, Wih, Wfh, Woh, bg, bi, bf, bo, Wph, bp,
           _s_total=S, _trace=False, _trace_kwargs=None):
    from concourse import bass_utils

    x = np.asarray(x, dtype=np.float32)
    args = [np.asarray(a, dtype=np.float32) for a in
            (Wgx, Wix, Wfx, Wox, Wgh, Wih, Wfh, Woh, bg, bi, bf, bo, Wph, bp)]
    Wx, Wh, Wp, bpr, Wx_all, b_all, Wh_all = _prep_shared_inputs(*args)

    nc = _get_nc(_s_total)
    in_maps = []
    for core in range(NCORES):
        m = _prep_core_inputs(x, Wx_all, b_all, Wh_all, Wp, bpr, core, _s_total)
        m.update({"Wx": Wx, "Wh": Wh, "Wp": Wp, "bp": bpr,
                  "h0": np.zeros((128, NK_H * BC), np.float32)})
        in_maps.append(m)

    kw = {}
    if _trace:
        kw["trace"] = True
        kw.update(_trace_kwargs or {})
    res = bass_utils.run_bass_kernel_spmd(nc, in_maps, core_ids=list(range(NCORES)), **kw)
    out = np.concatenate([res.results[c]["out"] for c in range(NCORES)], axis=0)
    if _trace:
        kernel._last_results = res
    return out


def _sim_selftest(s_total=16):
    """CoreSim numerics check on one core vs numpy LSTM (no hardware)."""
    from concourse.bass_interp import CoreSim

    rng = np.random.default_rng(0)
    x = rng.standard_normal((B, s_total, D), dtype=np.float32)
    mk = lambda *s: (rng.standard_normal(s, dtype=np.float32) * 0.06)
    Wgx, Wix, Wfx, Wox = (mk(D, H) for _ in range(4))
    Wgh, Wih, Wfh, Woh = (mk(H, H) for _ in range(4))
    bg, bi, bf, bo = (rng.standard_normal(H).astype(np.float32) * 0.05 for _ in range(4))
    Wph = mk(H, C)
    bp = rng.standard_normal(C).astype(np.float32) * 0.05

    def ref_np(xc):
        sig = lambda v: 1.0 / (1.0 + np.exp(-v))
        h = np.zeros((xc.shape[0], H), np.float32)
        c = np.zeros((xc.shape[0], H), np.float32)
        for t in range(s_total):
            xt = xc[:, t, :]
            g = np.tanh(xt @ Wgx + bg + h @ Wgh)
            i = sig(xt @ Wix + bi + h @ Wih)
            f = sig(xt @ Wfx + bf + h @ Wfh)
            o = sig(xt @ Wox + bo + h @ Woh)
            c = g * i + c * f
            h = np.tanh(c) * o
        return h @ Wph + bp

    args = (Wgx, Wix, Wfx, Wox, Wgh, Wih, Wfh, Woh, bg, bi, bf, bo, Wph, bp)
    Wx, Wh, Wp, bpr, Wx_all, b_all, Wh_all = _prep_shared_inputs(*args)
    nc = _build_nc(s_total)

    core = 1
    m = _prep_core_inputs(x, Wx_all, b_all, Wh_all, Wp, bpr, core, s_total)
    m.update({"Wx": Wx, "Wh": Wh, "Wp": Wp, "bp": bpr,
              "h0": np.zeros((128, NK_H * BC), np.float32)})

    sim = CoreSim(nc)
    for k, v in m.items():
        sim.tensor(k)[:] = v
    sim.simulate(check_with_hw=False)
    got = np.array(sim.tensor("out"))
    want = ref_np(x[core * BC:(core + 1) * BC])
    err = np.abs(got - want).max() / max(np.abs(want).max(), 1e-6)
    print(f"selftest S={s_total}: rel err {err:.3e}")
    assert err < 2e-2, err
    return err


if __name__ == "__main__":
    _sim_selftest(16)



# revision 2
# speedup vs baseline: 3.1671x; 3.1671x over previous
"""LSTM kernel for Trainium2 (Bass/Tile), SPMD over 8 NeuronCores — v2.

Problem: B=128, S=1024, D=256, H=512, C=10 LSTM; output = final hidden state
projected to C classes -> [B, C].

Sharding: data-parallel over batch (16 per core); weights replicated;
recurrence local per shard (no collectives).

v2 design (vs v1): everything transposed + fp16 GEMM path.
  * State kept transposed: hT/cT are [128 feature-partitions, 4*16] tiles
    (feature-tile k at cols 16k..16k+15, batch minor). No per-step PE
    transposes.
  * Recurrent GEMM is weight-stationary: lhsT = Wh tile [128, 128] fp16
    (FWL => ~32ns LDW+MM pair), rhs = hT [128, 16] moving. Pre-gates land
    transposed in PSUM, so all elementwise work runs on 128 partitions
    ([128, 64] tiles, ~135ns/op) instead of 16 ([16, 512], ~600ns/op).
  * Phase 1 (x @ Wx) fused into the recurrence: per 32-step block, 16
    chunk GEMMs write projT into SBUF (no DRAM round-trip); gate biases
    fold into the PSUM->SBUF move via tensor_scalar_add. Phase-1 PE work
    is interleaved one chunk per two steps to fill the PE idle tail.
  * Gate order per step: G,I,F first (12 chunks -> one [128,192] psum),
    O last (4 chunks -> [128,64] psum) so the c-update overlaps O's MMs.
"""

import numpy as np

S, B, D, H, C = 1024, 128, 256, 512, 10
NCORES = 8
BC = B // NCORES          # batch per core
TB = 32                   # timesteps per phase-1 block
NKH = H // 128            # 4 feature tiles for h
NKD = D // 128            # 2 feature tiles for x
NCH = 4 * NKH             # 16 gate chunks of 128 features (g-major: G,I,F,O)


def _build_nc(s_total: int):
    import concourse.bass as bass
    import concourse.mybir as mybir
    import concourse.tile as tile
    from concourse import bacc

    f32 = mybir.dt.float32
    f16 = mybir.dt.float16
    AF = mybir.ActivationFunctionType
    OP = mybir.AluOpType

    blocks = s_total // TB
    assert s_total % TB == 0

    nc = bacc.Bacc(
        "TRN2",
        target_bir_lowering=False,
        debug=False,
        enable_asserts=False,
        num_devices=NCORES,
    )

    xT_d = nc.dram_tensor("xT", [blocks, NKD, 128, TB * BC], f16, kind="ExternalInput").ap()
    Wh_d = nc.dram_tensor("Wh", [128, NCH * NKH * 128], f16, kind="ExternalInput").ap()
    Wx_d = nc.dram_tensor("Wx", [128, NCH * NKD * 128], f16, kind="ExternalInput").ap()
    b4_d = nc.dram_tensor("b4", [128, NCH], f32, kind="ExternalInput").ap()
    Wp_d = nc.dram_tensor("Wp", [128, NKH * C], f16, kind="ExternalInput").ap()
    bp_d = nc.dram_tensor("bp", [C, 1], f32, kind="ExternalInput").ap()
    outT_d = nc.dram_tensor("outT", [C, BC], f32, kind="ExternalOutput").ap()

    with tile.TileContext(nc) as tc:
        with (
            tc.tile_pool(name="const", bufs=1) as const,
            tc.tile_pool(name="state", bufs=1) as state,
            tc.tile_pool(name="xin", bufs=2) as xin,
            tc.tile_pool(name="ph1ps", bufs=2, space="PSUM") as ph1ps,
            tc.tile_pool(name="gifps", bufs=2, space="PSUM") as gifps,
            tc.tile_pool(name="ops", bufs=2, space="PSUM") as ops,
            tc.tile_pool(name="outps", bufs=1, space="PSUM") as outps,
            tc.tile_pool(name="gw", bufs=2) as gw,
        ):
            Wh_sb = const.tile([128, NCH * NKH * 128], f16)
            nc.sync.dma_start(Wh_sb[:], Wh_d[:])
            Wx_sb = const.tile([128, NCH * NKD * 128], f16)
            nc.sync.dma_start(Wx_sb[:], Wx_d[:])
            b4_sb = const.tile([128, NCH], f32)
            nc.sync.dma_start(b4_sb[:], b4_d[:])
            Wp_sb = const.tile([128, NKH * C], f16)
            nc.sync.dma_start(Wp_sb[:], Wp_d[:])
            bp_sb = const.tile([C, 1], f32)
            nc.sync.dma_start(bp_sb[:], bp_d[:])

            # Transposed recurrent state, ping-pong.
            hT = [state.tile([128, NKH * BC], f16, tag=f"hT{i}", name=f"hT{i}") for i in range(2)]
            cT = [state.tile([128, NKH * BC], f32, tag=f"cT{i}", name=f"cT{i}") for i in range(2)]
            nc.gpsimd.memset(hT[0][:], 0.0)
            nc.gpsimd.memset(cT[0][:], 0.0)

            # projT blocks: [128, chunk(16) x (toff*BC)] f32, ping-pong.
            proj = [state.tile([128, NCH * TB * BC], f32, tag=f"pj{i}", name=f"pj{i}")
                    for i in range(2)]
            xt_tiles = {}

            def dma_block(r):
                xt = xin.tile([128, NKD * TB * BC], f16, tag="xt", name="xt")
                nc.sync.dma_start(
                    xt[:].rearrange("p (k c) -> p k c", k=NKD),
                    xT_d[r].rearrange("k p c -> p k c"),
                )
                xt_tiles[r] = xt

            def phase1_chunk(r, c):
                xt = xt_tiles[r]
                pj = proj[r % 2]
                ps = ph1ps.tile([128, TB * BC], f32, tag="ph1", name="ph1")
                for d in range(NKD):
                    nc.tensor.matmul(
                        ps[:],
                        lhsT=Wx_sb[:, (c * NKD + d) * 128:(c * NKD + d + 1) * 128],
                        rhs=xt[:, d * TB * BC:(d + 1) * TB * BC],
                        start=(d == 0),
                        stop=(d == NKD - 1),
                    )
                nc.vector.tensor_scalar_add(
                    pj[:, c * TB * BC:(c + 1) * TB * BC], ps[:], b4_sb[:, c:c + 1])

            dma_block(0)
            for c in range(NCH):
                phase1_chunk(0, c)
            if blocks > 1:
                dma_block(1)

            for t in range(s_total):
                r, toff = divmod(t, TB)
                cur, nxt = t % 2, (t + 1) % 2
                if toff == 0 and r + 2 < blocks:
                    dma_block(r + 2)
                if t % 2 == 0 and r + 1 < blocks:
                    phase1_chunk(r + 1, toff // 2)

                pj3 = proj[r % 2][:].rearrange("p (c w) -> p c w", c=NCH)
                sl = slice(toff * BC, (toff + 1) * BC)

                # G, I, F pre-gates: 12 chunks -> one [128, 192] psum tile.
                psG = gifps.tile([128, 12 * BC], f32, tag="gif", name="gif")
                for c in range(12):
                    for k in range(NKH):
                        nc.tensor.matmul(
                            psG[:, c * BC:(c + 1) * BC],
                            lhsT=Wh_sb[:, (c * NKH + k) * 128:(c * NKH + k + 1) * 128],
                            rhs=hT[cur][:, k * BC:(k + 1) * BC],
                            start=(k == 0),
                            stop=(k == NKH - 1),
                        )
                pre = gw.tile([128, 12 * BC], f32, tag="pre", name="pre")
                nc.vector.scalar_tensor_tensor(
                    pre[:].rearrange("p (c w) -> p c w", c=12),
                    psG[:].rearrange("p (c w) -> p c w", c=12),
                    1.0,
                    pj3[:, 0:12, sl],
                    op0=OP.mult,
                    op1=OP.add,
                )
                gt = gw.tile([128, 4 * BC], f32, tag="gt", name="gt")
                nc.scalar.activation(gt[:], pre[:, 0:4 * BC], AF.Tanh)
                it = gw.tile([128, 4 * BC], f32, tag="it", name="it")
                nc.scalar.activation(it[:], pre[:, 4 * BC:8 * BC], AF.Sigmoid)
                ft = gw.tile([128, 4 * BC], f32, tag="ft", name="ft")
                nc.scalar.activation(ft[:], pre[:, 8 * BC:12 * BC], AF.Sigmoid)

                gi = gw.tile([128, 4 * BC], f32, tag="gi", name="gi")
                nc.vector.tensor_mul(gi[:], gt[:], it[:])
                cn = cT[nxt]
                nc.vector.tensor_mul(cn[:], cT[cur][:], ft[:])
                nc.vector.tensor_add(cn[:], cn[:], gi[:])
                th = gw.tile([128, 4 * BC], f32, tag="th", name="th")
                nc.scalar.activation(th[:], cn[:], AF.Tanh)

                # O pre-gate: 4 chunks -> [128, 64] psum tile.
                psO = ops.tile([128, 4 * BC], f32, tag="po", name="po")
                for c4 in range(4):
                    c = 12 + c4
                    for k in range(NKH):
                        nc.tensor.matmul(
                            psO[:, c4 * BC:(c4 + 1) * BC],
                            lhsT=Wh_sb[:, (c * NKH + k) * 128:(c * NKH + k + 1) * 128],
                            rhs=hT[cur][:, k * BC:(k + 1) * BC],
                            start=(k == 0),
                            stop=(k == NKH - 1),
                        )
                preO = gw.tile([128, 4 * BC], f32, tag="preO", name="preO")
                nc.vector.scalar_tensor_tensor(
                    preO[:].rearrange("p (c w) -> p c w", c=4),
                    psO[:].rearrange("p (c w) -> p c w", c=4),
                    1.0,
                    pj3[:, 12:16, sl],
                    op0=OP.mult,
                    op1=OP.add,
                )
                ot = gw.tile([128, 4 * BC], f32, tag="ot", name="ot")
                nc.scalar.activation(ot[:], preO[:], AF.Sigmoid)
                nc.vector.tensor_mul(hT[nxt][:], th[:], ot[:])

            # Final projection: outT = Wp.T @ h_S + bp  -> [C, BC]
            fin = s_total % 2
            pso = outps.tile([C, BC], f32, tag="pout", name="pout")
            for k in range(NKH):
                nc.tensor.matmul(
                    pso[:],
                    lhsT=Wp_sb[:, k * C:(k + 1) * C],
                    rhs=hT[fin][:, k * BC:(k + 1) * BC],
                    start=(k == 0),
                    stop=(k == NKH - 1),
                )
            res = gw.tile([C, BC], f32, tag="res", name="res")
            nc.vector.tensor_scalar_add(res[:], pso[:], bp_sb[:, 0:1])
            nc.sync.dma_start(outT_d[:], res[:])

    nc.compile()
    return nc


def _prep_shared_inputs(Wgx, Wix, Wfx, Wox, Wgh, Wih, Wfh, Woh, bg, bi, bf, bo, Wph, bp):
    Wx_all = np.concatenate([Wgx, Wix, Wfx, Wox], axis=1).astype(np.float32)  # [D, G4]
    b_all = np.concatenate([bg, bi, bf, bo]).astype(np.float32)               # [G4]
    Wh_all = np.concatenate([Wgh, Wih, Wfh, Woh], axis=1).astype(np.float32)  # [H, G4]

    # Wh_sb[p, ((c*NKH)+kin)*128 + j] = Wh_all[kin*128+p, c*128+j], c = g*4+kout
    Wh = Wh_all.reshape(NKH, 128, NCH, 128).transpose(1, 2, 0, 3).reshape(128, NCH * NKH * 128)
    Wx = Wx_all.reshape(NKD, 128, NCH, 128).transpose(1, 2, 0, 3).reshape(128, NCH * NKD * 128)
    b4 = b_all.reshape(NCH, 128).transpose(1, 0).copy()                       # [128, 16]
    Wp = Wph.astype(np.float32).reshape(NKH, 128, C).transpose(1, 0, 2).reshape(128, NKH * C)
    bpc = bp.astype(np.float32).reshape(C, 1).copy()
    return (np.ascontiguousarray(Wh).astype(np.float16),
            np.ascontiguousarray(Wx).astype(np.float16),
            np.ascontiguousarray(b4),
            np.ascontiguousarray(Wp).astype(np.float16),
            bpc)


def _prep_core_x(x, core, s_total):
    blocks = s_total // TB
    b0 = core * BC
    xc = np.asarray(x[b0:b0 + BC, :s_total, :], dtype=np.float16)   # [BC, s, D]
    # xT[r, k, p, toff*BC + b] = xc[b, r*TB+toff, k*128+p]
    a = xc.transpose(2, 1, 0)                                       # [D, s, BC]
    a = a.reshape(NKD, 128, blocks, TB, BC)
    a = a.transpose(2, 0, 1, 3, 4).reshape(blocks, NKD, 128, TB * BC)
    return np.ascontiguousarray(a)


_NC_CACHE = {}


def _get_nc(s_total):
    if s_total not in _NC_CACHE:
        _NC_CACHE[s_total] = _build_nc(s_total)
    return _NC_CACHE[s_total]


def kernel(x, Wgx, Wix, Wfx, Wox, Wgh, Wih, Wfh, Woh, bg, bi, bf, bo, Wph, bp,
           _s_total=S, _trace=False, _trace_kwargs=None):
    from concourse import bass_utils

    x = np.asarray(x, dtype=np.float32)
    args = [np.asarray(a, dtype=np.float32) for a in
            (Wgx, Wix, Wfx, Wox, Wgh, Wih, Wfh, Woh, bg, bi, bf, bo, Wph, bp)]
    Wh, Wx, b4, Wp, bpc = _prep_shared_inputs(*args)

    nc = _get_nc(_s_total)
    in_maps = []
    for core in range(NCORES):
        in_maps.append({
            "xT": _prep_core_x(x, core, _s_total),
            "Wh": Wh, "Wx": Wx, "b4": b4, "Wp": Wp, "bp": bpc,
        })

    kw = {}
    if _trace:
        kw["trace"] = True
        kw.update(_trace_kwargs or {})
    res = bass_utils.run_bass_kernel_spmd(nc, in_maps, core_ids=list(range(NCORES)), **kw)
    out = np.concatenate(
        [res.results[c]["outT"].T for c in range(NCORES)], axis=0).astype(np.float32)
    if _trace:
        kernel._last_results = res
    return np.ascontiguousarray(out)


def _sim_selftest(s_total=32):
    """CoreSim numerics check on one core vs numpy LSTM (no hardware)."""
    from concourse.bass_interp import CoreSim

    rng = np.random.default_rng(0)
    x = rng.standard_normal((B, s_total, D), dtype=np.float32)
    mk = lambda *s: (rng.standard_normal(s, dtype=np.float32) * 0.06)
    Wgx, Wix, Wfx, Wox = (mk(D, H) for _ in range(4))
    Wgh, Wih, Wfh, Woh = (mk(H, H) for _ in range(4))
    bg, bi, bf, bo = (rng.standard_normal(H).astype(np.float32) * 0.05 for _ in range(4))
    Wph = mk(H, C)
    bp = rng.standard_normal(C).astype(np.float32) * 0.05

    def ref_np(xc):
        sig = lambda v: 1.0 / (1.0 + np.exp(-v))
        h = np.zeros((xc.shape[0], H), np.float32)
        c = np.zeros((xc.shape[0], H), np.float32)
        for t in range(s_total):
            xt = xc[:, t, :]
            g = np.tanh(xt @ Wgx + bg + h @ Wgh)
            i = sig(xt @ Wix + bi + h @ Wih)
            f = sig(xt @ Wfx + bf + h @ Wfh)
            o = sig(xt @ Wox + bo + h @ Woh)
            c = g * i + c * f
            h = np.tanh(c) * o
        return h @ Wph + bp

    args = (Wgx, Wix, Wfx, Wox, Wgh, Wih, Wfh, Woh, bg, bi, bf, bo, Wph, bp)
    Wh, Wx, b4, Wp, bpc = _prep_shared_inputs(*args)
    nc = _build_nc(s_total)

    core = 1
    m = {"xT": _prep_core_x(x, core, s_total),
         "Wh": Wh, "Wx": Wx, "b4": b4, "Wp": Wp, "bp": bpc}

    sim = CoreSim(nc)
    for k, v in m.items():
        sim.tensor(k)[:] = v
    sim.simulate(check_with_hw=False)
    got = np.array(sim.tensor("outT")).T
    want = ref_np(x[core * BC:(core + 1) * BC])
    err = np.abs(got - want).max() / max(np.abs(want).max(), 1e-6)
    print(f"selftest S={s_total}: rel err {err:.3e}")
    assert err < 2e-2, err
    return err


if __name__ == "__main__":
    _sim_selftest(32)


# revision 3
# speedup vs baseline: 3.5175x; 1.1106x over previous
"""LSTM kernel for Trainium2 (Bass/Tile), SPMD over 8 NeuronCores — v2.

Problem: B=128, S=1024, D=256, H=512, C=10 LSTM; output = final hidden state
projected to C classes -> [B, C].

Sharding: data-parallel over batch (16 per core); weights replicated;
recurrence local per shard (no collectives).

v2 design (vs v1): everything transposed + fp16 GEMM path.
  * State kept transposed: hT/cT are [128 feature-partitions, 4*16] tiles
    (feature-tile k at cols 16k..16k+15, batch minor). No per-step PE
    transposes.
  * Recurrent GEMM is weight-stationary: lhsT = Wh tile [128, 128] fp16
    (FWL => ~32ns LDW+MM pair), rhs = hT [128, 16] moving. Pre-gates land
    transposed in PSUM, so all elementwise work runs on 128 partitions
    ([128, 64] tiles, ~135ns/op) instead of 16 ([16, 512], ~600ns/op).
  * Phase 1 (x @ Wx) fused into the recurrence: per 32-step block, 16
    chunk GEMMs write projT into SBUF (no DRAM round-trip); gate biases
    fold into the PSUM->SBUF move via tensor_scalar_add. Phase-1 PE work
    is interleaved one chunk per two steps to fill the PE idle tail.
  * Gate order per step: G,I,F first (12 chunks -> one [128,192] psum),
    O last (4 chunks -> [128,64] psum) so the c-update overlaps O's MMs.
"""

import numpy as np

S, B, D, H, C = 1024, 128, 256, 512, 10
NCORES = 8
BC = B // NCORES          # batch per core
TB = 32                   # timesteps per phase-1 block
NKH = H // 128            # 4 feature tiles for h
NKD = D // 128            # 2 feature tiles for x
NCH = 4 * NKH             # 16 gate chunks of 128 features (g-major: G,I,F,O)


def _build_nc(s_total: int):
    import concourse.bass as bass
    import concourse.mybir as mybir
    import concourse.tile as tile
    from concourse import bacc

    f32 = mybir.dt.float32
    f16 = mybir.dt.float16
    AF = mybir.ActivationFunctionType
    OP = mybir.AluOpType

    blocks = s_total // TB
    assert s_total % TB == 0

    nc = bacc.Bacc(
        "TRN2",
        target_bir_lowering=False,
        debug=False,
        enable_asserts=False,
        num_devices=NCORES,
    )

    xT_d = nc.dram_tensor("xT", [blocks, NKD, 128, TB * BC], f16, kind="ExternalInput").ap()
    Wh_d = nc.dram_tensor("Wh", [128, NCH * NKH * 128], f16, kind="ExternalInput").ap()
    Wx_d = nc.dram_tensor("Wx", [128, NCH * NKD * 128], f16, kind="ExternalInput").ap()
    b4_d = nc.dram_tensor("b4", [128, NCH], f32, kind="ExternalInput").ap()
    Wp_d = nc.dram_tensor("Wp", [128, NKH * C], f16, kind="ExternalInput").ap()
    bp_d = nc.dram_tensor("bp", [C, 1], f32, kind="ExternalInput").ap()
    outT_d = nc.dram_tensor("outT", [C, BC], f32, kind="ExternalOutput").ap()

    with tile.TileContext(nc) as tc:
        with (
            tc.tile_pool(name="const", bufs=1) as const,
            tc.tile_pool(name="state", bufs=1) as state,
            tc.tile_pool(name="xin", bufs=2) as xin,
            tc.tile_pool(name="ph1ps", bufs=2, space="PSUM") as ph1ps,
            tc.tile_pool(name="gifps", bufs=5, space="PSUM") as gifps,
            tc.tile_pool(name="outps", bufs=1, space="PSUM") as outps,
            tc.tile_pool(name="gw", bufs=2) as gw,
        ):
            Wh_sb = const.tile([128, NCH * NKH * 128], f16)
            nc.sync.dma_start(Wh_sb[:], Wh_d[:])
            Wx_sb = const.tile([128, NCH * NKD * 128], f16)
            nc.sync.dma_start(Wx_sb[:], Wx_d[:])
            b4_sb = const.tile([128, NCH], f32)
            nc.sync.dma_start(b4_sb[:], b4_d[:])
            Wp_sb = const.tile([128, NKH * C], f16)
            nc.sync.dma_start(Wp_sb[:], Wp_d[:])
            bp_sb = const.tile([C, 1], f32)
            nc.sync.dma_start(bp_sb[:], bp_d[:])

            # Transposed recurrent state, ping-pong.
            hT = [state.tile([128, NKH * BC], f16, tag=f"hT{i}", name=f"hT{i}") for i in range(2)]
            cT = [state.tile([128, NKH * BC], f32, tag=f"cT{i}", name=f"cT{i}") for i in range(2)]
            nc.gpsimd.memset(hT[0][:], 0.0)
            nc.gpsimd.memset(cT[0][:], 0.0)

            # projT blocks: [128, chunk(16) x (toff*BC)] f32, ping-pong.
            proj = [state.tile([128, NCH * TB * BC], f32, tag=f"pj{i}", name=f"pj{i}")
                    for i in range(2)]
            xt_tiles = {}

            def dma_block(r):
                xt = xin.tile([128, NKD * TB * BC], f16, tag="xt", name="xt")
                nc.sync.dma_start(
                    xt[:].rearrange("p (k c) -> p k c", k=NKD),
                    xT_d[r].rearrange("k p c -> p k c"),
                )
                xt_tiles[r] = xt

            def phase1_mm(r, c):
                xt = xt_tiles[r]
                ps = ph1ps.tile([128, TB * BC], f32, tag="ph1", name=f"ph1_{r}_{c}")
                for d in range(NKD):
                    nc.tensor.matmul(
                        ps[:],
                        lhsT=Wx_sb[:, (c * NKD + d) * 128:(c * NKD + d + 1) * 128],
                        rhs=xt[:, d * TB * BC:(d + 1) * TB * BC],
                        start=(d == 0),
                        stop=(d == NKD - 1),
                    )
                return ps

            def phase1_store(r, c, ps):
                # Emitted late so the DVE runs it in its idle window during
                # the next step's MM stream, off the critical chain.
                pj = proj[r % 2]
                nc.vector.tensor_scalar_add(
                    pj[:, c * TB * BC:(c + 1) * TB * BC], ps[:], b4_sb[:, c:c + 1])

            def phase1_chunk(r, c):
                phase1_store(r, c, phase1_mm(r, c))

            dma_block(0)
            for c in range(NCH):
                phase1_chunk(0, c)
            if blocks > 1:
                dma_block(1)

            for t in range(s_total):
                r, toff = divmod(t, TB)
                cur, nxt = t % 2, (t + 1) % 2
                if toff == 0 and r + 2 < blocks:
                    dma_block(r + 2)
                ph1 = None
                if t % 2 == 0 and r + 1 < blocks:
                    ph1 = (r + 1, toff // 2, phase1_mm(r + 1, toff // 2))

                pj3 = proj[r % 2][:].rearrange("p (c w) -> p c w", c=NCH)
                sl = slice(toff * BC, (toff + 1) * BC)

                # Pre-gates: one [128, 64] psum tile per gate (G, I, F, O),
                # each 16 MM pairs, so gate g's elementwise chain starts as
                # soon as its quarter of the MM stream completes.
                psg = []
                for g in range(4):
                    ps = gifps.tile([128, 4 * BC], f32, tag="pg", name=f"pg{g}")
                    for c4 in range(NKH):
                        c = g * NKH + c4
                        for k in range(NKH):
                            nc.tensor.matmul(
                                ps[:, c4 * BC:(c4 + 1) * BC],
                                lhsT=Wh_sb[:, (c * NKH + k) * 128:(c * NKH + k + 1) * 128],
                                rhs=hT[cur][:, k * BC:(k + 1) * BC],
                                start=(k == 0),
                                stop=(k == NKH - 1),
                            )
                    psg.append(ps)

                def pre_gate(g):
                    pre = gw.tile([128, 4 * BC], f32, tag=f"pre{g}", name=f"pre{g}")
                    nc.vector.scalar_tensor_tensor(
                        pre[:].rearrange("p (c w) -> p c w", c=NKH),
                        psg[g][:].rearrange("p (c w) -> p c w", c=NKH),
                        1.0,
                        pj3[:, g * NKH:(g + 1) * NKH, sl],
                        op0=OP.mult,
                        op1=OP.add,
                    )
                    return pre

                # Emission order fixes each engine's FIFO:
                #   DVE: stt_G, stt_I, stt_F, gi, stt_O, cf, c', h01, h23
                #   ACT: tanhG, sigI, sigF, sigO, th
                preG = pre_gate(0)
                gt = gw.tile([128, 4 * BC], f32, tag="gt", name="gt")
                nc.scalar.activation(gt[:], preG[:], AF.Tanh)
                preI = pre_gate(1)
                it = gw.tile([128, 4 * BC], f32, tag="it", name="it")
                nc.scalar.activation(it[:], preI[:], AF.Sigmoid)
                preF = pre_gate(2)
                ft = gw.tile([128, 4 * BC], f32, tag="ft", name="ft")
                nc.scalar.activation(ft[:], preF[:], AF.Sigmoid)
                gi = gw.tile([128, 4 * BC], f32, tag="gi", name="gi")
                nc.vector.tensor_mul(gi[:], gt[:], it[:])
                preO = pre_gate(3)
                ot = gw.tile([128, 4 * BC], f32, tag="ot", name="ot")
                nc.scalar.activation(ot[:], preO[:], AF.Sigmoid)
                cn = cT[nxt]
                nc.vector.tensor_mul(cn[:], cT[cur][:], ft[:])
                nc.vector.tensor_add(cn[:], cn[:], gi[:])
                th = gw.tile([128, 4 * BC], f32, tag="th", name="th")
                nc.scalar.activation(th[:], cn[:], AF.Tanh)
                nc.vector.tensor_mul(hT[nxt][:, 0:2 * BC], th[:, 0:2 * BC], ot[:, 0:2 * BC])
                nc.vector.tensor_mul(hT[nxt][:, 2 * BC:4 * BC], th[:, 2 * BC:4 * BC], ot[:, 2 * BC:4 * BC])

                if ph1 is not None:
                    phase1_store(*ph1)

            # Final projection: outT = Wp.T @ h_S + bp  -> [C, BC]
            fin = s_total % 2
            pso = outps.tile([C, BC], f32, tag="pout", name="pout")
            for k in range(NKH):
                nc.tensor.matmul(
                    pso[:],
                    lhsT=Wp_sb[:, k * C:(k + 1) * C],
                    rhs=hT[fin][:, k * BC:(k + 1) * BC],
                    start=(k == 0),
                    stop=(k == NKH - 1),
                )
            res = gw.tile([C, BC], f32, tag="res", name="res")
            nc.vector.tensor_scalar_add(res[:], pso[:], bp_sb[:, 0:1])
            nc.sync.dma_start(outT_d[:], res[:])

    nc.compile()
    return nc


def _prep_shared_inputs(Wgx, Wix, Wfx, Wox, Wgh, Wih, Wfh, Woh, bg, bi, bf, bo, Wph, bp):
    Wx_all = np.concatenate([Wgx, Wix, Wfx, Wox], axis=1).astype(np.float32)  # [D, G4]
    b_all = np.concatenate([bg, bi, bf, bo]).astype(np.float32)               # [G4]
    Wh_all = np.concatenate([Wgh, Wih, Wfh, Woh], axis=1).astype(np.float32)  # [H, G4]

    # Wh_sb[p, ((c*NKH)+kin)*128 + j] = Wh_all[kin*128+p, c*128+j], c = g*4+kout
    Wh = Wh_all.reshape(NKH, 128, NCH, 128).transpose(1, 2, 0, 3).reshape(128, NCH * NKH * 128)
    Wx = Wx_all.reshape(NKD, 128, NCH, 128).transpose(1, 2, 0, 3).reshape(128, NCH * NKD * 128)
    b4 = b_all.reshape(NCH, 128).transpose(1, 0).copy()                       # [128, 16]
    Wp = Wph.astype(np.float32).reshape(NKH, 128, C).transpose(1, 0, 2).reshape(128, NKH * C)
    bpc = bp.astype(np.float32).reshape(C, 1).copy()
    return (np.ascontiguousarray(Wh).astype(np.float16),
            np.ascontiguousarray(Wx).astype(np.float16),
            np.ascontiguousarray(b4),
            np.ascontiguousarray(Wp).astype(np.float16),
            bpc)


def _prep_core_x(x, core, s_total):
    blocks = s_total // TB
    b0 = core * BC
    xc = np.asarray(x[b0:b0 + BC, :s_total, :], dtype=np.float16)   # [BC, s, D]
    # xT[r, k, p, toff*BC + b] = xc[b, r*TB+toff, k*128+p]
    a = xc.transpose(2, 1, 0)                                       # [D, s, BC]
    a = a.reshape(NKD, 128, blocks, TB, BC)
    a = a.transpose(2, 0, 1, 3, 4).reshape(blocks, NKD, 128, TB * BC)
    return np.ascontiguousarray(a)


_NC_CACHE = {}


def _get_nc(s_total):
    if s_total not in _NC_CACHE:
        _NC_CACHE[s_total] = _build_nc(s_total)
    return _NC_CACHE[s_total]


def kernel(x, Wgx, Wix, Wfx, Wox, Wgh, Wih, Wfh, Woh, bg, bi, bf, bo, Wph, bp,
           _s_total=S, _trace=False, _trace_kwargs=None):
    from concourse import bass_utils

    x = np.asarray(x, dtype=np.float32)
    args = [np.asarray(a, dtype=np.float32) for a in
            (Wgx, Wix, Wfx, Wox, Wgh, Wih, Wfh, Woh, bg, bi, bf, bo, Wph, bp)]
    Wh, Wx, b4, Wp, bpc = _prep_shared_inputs(*args)

    nc = _get_nc(_s_total)
    in_maps = []
    for core in range(NCORES):
        in_maps.append({
            "xT": _prep_core_x(x, core, _s_total),
            "Wh": Wh, "Wx": Wx, "b4": b4, "Wp": Wp, "bp": bpc,
        })

    kw = {}
    if _trace:
        kw["trace"] = True
        kw.update(_trace_kwargs or {})
    res = bass_utils.run_bass_kernel_spmd(nc, in_maps, core_ids=list(range(NCORES)), **kw)
    out = np.concatenate(
        [res.results[c]["outT"].T for c in range(NCORES)], axis=0).astype(np.float32)
    if _trace:
        kernel._last_results = res
    return np.ascontiguousarray(out)


def _sim_selftest(s_total=32):
    """CoreSim numerics check on one core vs numpy LSTM (no hardware)."""
    from concourse.bass_interp import CoreSim

    rng = np.random.default_rng(0)
    x = rng.standard_normal((B, s_total, D), dtype=np.float32)
    mk = lambda *s: (rng.standard_normal(s, dtype=np.float32) * 0.06)
    Wgx, Wix, Wfx, Wox = (mk(D, H) for _ in range(4))
    Wgh, Wih, Wfh, Woh = (mk(H, H) for _ in range(4))
    bg, bi, bf, bo = (rng.standard_normal(H).astype(np.float32) * 0.05 for _ in range(4))
    Wph = mk(H, C)
    bp = rng.standard_normal(C).astype(np.float32) * 0.05

    def ref_np(xc):
        sig = lambda v: 1.0 / (1.0 + np.exp(-v))
        h = np.zeros((xc.shape[0], H), np.float32)
        c = np.zeros((xc.shape[0], H), np.float32)
        for t in range(s_total):
            xt = xc[:, t, :]
            g = np.tanh(xt @ Wgx + bg + h @ Wgh)
            i = sig(xt @ Wix + bi + h @ Wih)
            f = sig(xt @ Wfx + bf + h @ Wfh)
            o = sig(xt @ Wox + bo + h @ Woh)
            c = g * i + c * f
            h = np.tanh(c) * o
        return h @ Wph + bp

    args = (Wgx, Wix, Wfx, Wox, Wgh, Wih, Wfh, Woh, bg, bi, bf, bo, Wph, bp)
    Wh, Wx, b4, Wp, bpc = _prep_shared_inputs(*args)
    nc = _build_nc(s_total)

    core = 1
    m = {"xT": _prep_core_x(x, core, s_total),
         "Wh": Wh, "Wx": Wx, "b4": b4, "Wp": Wp, "bp": bpc}

    sim = CoreSim(nc)
    for k, v in m.items():
        sim.tensor(k)[:] = v
    sim.simulate(check_with_hw=False)
    got = np.array(sim.tensor("outT")).T
    want = ref_np(x[core * BC:(core + 1) * BC])
    err = np.abs(got - want).max() / max(np.abs(want).max(), 1e-6)
    print(f"selftest S={s_total}: rel err {err:.3e}")
    assert err < 2e-2, err
    return err


if __name__ == "__main__":
    _sim_selftest(32)
